# revision 1
# baseline (speedup 1.0000x reference)
"""Trainium2 Bass kernel for nn_MeshNodeBlock (GNN message passing block).

reference semantics:
    agg = segment_sum(edge_features, src_indices, N)        # scatter-add
    x   = concat([node_features, agg], -1)
    h   = silu(x @ W1 + b1)
    y   = h @ W2 + b2
    y   = layer_norm(y) * gamma + beta
    out = y + node_features

Strategy (8 NeuronCores, SPMD, one NEFF):
  * Host graph-partitions nodes contiguously across cores (12800 node slots
    per core) and stable-sorts edges by destination node; each core receives
    exactly the edge rows destined for its nodes, grouped by 128-node tile
    and padded to a per-tile-position chunk count C_i (shared across cores
    so the SPMD program is uniform; pad rows are zero).
  * Device works fully in transposed space (features on partitions, nodes on
    free dim). Per 128-node tile the scatter-add is C_i PE matmuls
    aggT += edge_chunk.T @ onehot. One-hot blocks for a whole tile are built
    in one 2x-mode vector is_equal against a tiled-iota constant, with the
    local ids pre-expanded by a gpsimd broadcast copy.
  * MLP consumes aggT/nodeT directly: layer 1 -> hT_j slices, silu(+b1) on
    the scalar engine, layer 2 -> yT.
  * LayerNorm stats via matmuls whose lhsT is a block-diagonal 1/128 column
    (ONCB): group g's mean/mean-of-squares land on PSUM row g of a shared
    bank, accumulated over a block of groups. Stats post-processing
    (var, rstd=exp(-0.5*ln(var+eps))) runs once per block at full width,
    then rows bounce through a DRAM tile and DMA-broadcast back across
    partitions. Processing is phase-blocked to minimize ACT table switches.
  * Output written transposed in bf16; host transposes/casts back.
"""

import functools
from contextlib import ExitStack

import numpy as np
import ml_dtypes

import concourse.bass as bass
import concourse.tile as tile
from concourse import bacc, mybir
from concourse import bass_utils

BF16 = ml_dtypes.bfloat16
FP8 = ml_dtypes.float8_e4m3

N_NODES = 100000
D = 128
N_CORES = 8
P = 128
GROUP = 512              # nodes per group = 4 tiles
NODES_PER_CORE = 12800   # 25 groups
C_MAX = 8                # fallback chunk budget per tile (exact counts used)
NBLK = 2                 # phase blocks
INTERLEAVE_P3 = True    # interleave prev block's normalize into next phase1
EPS = 1e-5

AF = mybir.ActivationFunctionType
ALU = mybir.AluOpType
dt = mybir.dt


# --------------------------------------------------------------------------
# device kernel builder
# --------------------------------------------------------------------------

@functools.lru_cache(maxsize=4)
def _build(nodes_per_core: int, cis: tuple, n_cores: int, act: str = "silu"):
    assert nodes_per_core % GROUP == 0
    n_groups = nodes_per_core // GROUP
    tiles_per_core = nodes_per_core // P
    assert len(cis) == tiles_per_core
    coff = np.concatenate([[0], np.cumsum(cis)]).astype(int)
    ch = int(coff[-1])                   # total chunks per core
    cmaxt = int(max(cis))
    gbytes_max = max(
        (int(coff[gg * 4 + 4]) - int(coff[gg * 4])) * 384
        for gg in range(nodes_per_core // GROUP))

    # phase blocks of groups (ACT table switches cost ~2.7us per set swap).
    # Asymmetric: big first block, small last block whose normalize tail is
    # all that remains after PE finishes.
    if n_groups >= 8:
        ntail = 4
        blocks = [list(range(0, n_groups - ntail)),
                  list(range(n_groups - ntail, n_groups))]
    else:
        blocks = [list(range(n_groups))]
    bmax = max(len(b) for b in blocks)

    nc = bacc.Bacc("TRN2", target_bir_lowering=False, debug=False,
                   enable_asserts=False, num_devices=n_cores)

    PK = nc.dram_tensor("pk", [P, ch * 384], dt.uint8, kind="ExternalInput").ap()
    NTB = nc.dram_tensor("ntb", [P, nodes_per_core], dt.bfloat16,
                         kind="ExternalInput").ap()
    W1P = nc.dram_tensor("w1p", [P, 1024], dt.bfloat16, kind="ExternalInput").ap()
    W2P = nc.dram_tensor("w2p", [P, 512], dt.bfloat16, kind="ExternalInput").ap()
    B1P = nc.dram_tensor("b1p", [P, 4], dt.float32, kind="ExternalInput").ap()
    B2P = nc.dram_tensor("b2p", [P, 1], dt.float32, kind="ExternalInput").ap()
    GAM = nc.dram_tensor("gam", [P, 1], dt.float32, kind="ExternalInput").ap()
    BET = nc.dram_tensor("bet", [P, 1], dt.float32, kind="ExternalInput").ap()
    ONB = nc.dram_tensor("onb", [P, bmax * 128], dt.bfloat16,
                         kind="ExternalInput").ap()
    OUT = nc.dram_tensor("out", [P, nodes_per_core], dt.bfloat16,
                         kind="ExternalOutput").ap()

    with tile.TileContext(nc) as tc:
        with ExitStack() as ctx:
            singles = ctx.enter_context(tc.tile_pool(name="singles", bufs=1))
            ebp = ctx.enter_context(tc.tile_pool(name="ebp", bufs=4))
            xtp = ctx.enter_context(tc.tile_pool(name="xtp", bufs=n_groups + 2))
            xap = ctx.enter_context(tc.tile_pool(name="xap", bufs=4))
            shp = ctx.enter_context(tc.tile_pool(name="shp", bufs=2))
            yp = ctx.enter_context(tc.tile_pool(name="yp", bufs=n_groups + 2))
            y2p = ctx.enter_context(tc.tile_pool(name="y2p", bufs=bmax + 2))
            zp = ctx.enter_context(tc.tile_pool(name="zp", bufs=6))
            stp = ctx.enter_context(tc.tile_pool(name="stp", bufs=1))
            psagg = ctx.enter_context(tc.tile_pool(name="psagg", bufs=2, space="PSUM"))
            psh = ctx.enter_context(tc.tile_pool(name="psh", bufs=3, space="PSUM"))
            psy = ctx.enter_context(tc.tile_pool(name="psy", bufs=1, space="PSUM"))
            psst = ctx.enter_context(tc.tile_pool(name="psst", bufs=1, space="PSUM"))
            drp = ctx.enter_context(tc.tile_pool(name="drp", bufs=2, space="DRAM"))

            def load_const(name, src, shape, dtyp):
                t = singles.tile(shape, dtyp, tag=name)
                nc.sync.dma_start(out=t[:], in_=src)
                return t

            w1 = load_const("w1", W1P, [P, 1024], dt.bfloat16)
            w2 = load_const("w2", W2P, [P, 512], dt.bfloat16)
            b1 = load_const("b1", B1P, [P, 4], dt.float32)
            b2 = load_const("b2", B2P, [P, 1], dt.float32)
            gam = load_const("gam", GAM, [P, 1], dt.float32)
            bet = load_const("bet", BET, [P, 1], dt.float32)
            onb = load_const("onb", ONB, [P, bmax * 128], dt.bfloat16)
            eps = singles.tile([P, 1], dt.float32, tag="eps")
            nc.vector.memset(eps[:], EPS)

            y_tiles = {}
            y2_tiles = {}
            _last_stats = []

            xta_tiles = {}
            xtn_tiles = {}

            def phase1(block, bi, interleave=None):
                bsz = len(block)
                mu_ps = psst.tile([P, GROUP], dt.float32, tag="mups")
                m2_ps = psst.tile([P, GROUP], dt.float32, tag="m2ps")
                for gi, g in enumerate(block):
                    nsl = slice(g * GROUP, (g + 1) * GROUP)
                    xtn = xtp.tile([P, GROUP], dt.bfloat16, tag="xtn")
                    nc.sync.dma_start(out=xtn[:], in_=NTB[:, nsl])
                    xtn_tiles[g] = xtn

                    agg_ps = psagg.tile([P, GROUP], dt.float32, tag="agg")
                    g0 = int(coff[g * 4])
                    gbytes = (int(coff[g * 4 + 4]) - g0) * 384
                    pk = ebp.tile([P, gbytes_max], dt.uint8, tag="pk")
                    nc.sync.dma_start(out=pk[:, :gbytes],
                                      in_=PK[:, g0 * 384:g0 * 384 + gbytes])
                    for t4 in range(4):
                        ti = g * 4 + t4
                        ci = int(cis[ti])
                        toff = (int(coff[ti]) - g0) * 384
                        ebv = pk[:, toff:toff + ci * 256].bitcast(dt.bfloat16)
                        ohv = pk[:, toff + ci * 256:toff + ci * 384].bitcast(
                            dt.float8e4)
                        for c in range(ci):
                            nc.tensor.matmul(
                                out=agg_ps[:, t4 * 128:(t4 + 1) * 128],
                                lhsT=ebv[:, c * 128:(c + 1) * 128],
                                rhs=ohv[:, c * 128:(c + 1) * 128],
                                start=(c == 0), stop=(c == ci - 1))
                    xta = xap.tile([P, GROUP], dt.bfloat16, tag="xta")
                    if g % 2 == 0:
                        nc.scalar.activation(out=xta[:], in_=agg_ps[:], func=AF.Copy)
                    else:
                        nc.vector.tensor_copy(out=xta[:], in_=agg_ps[:])
                    sh_tiles = []
                    for j in range(4):
                        hps = psh.tile([P, GROUP], dt.float32, tag="hps")
                        nc.tensor.matmul(out=hps[:],
                                         lhsT=w1[:, j * 128:(j + 1) * 128],
                                         rhs=xtn[:], start=True, stop=False)
                        nc.tensor.matmul(
                            out=hps[:],
                            lhsT=w1[:, 512 + j * 128:512 + (j + 1) * 128],
                            rhs=xta[:], start=False, stop=True)
                        sh = shp.tile([P, GROUP], dt.bfloat16, tag=f"sh{j}")
                        if act == "silu":
                            nc.scalar.activation(out=sh[:], in_=hps[:],
                                                 func=AF.Silu,
                                                 bias=b1[:, j:j + 1], scale=1.0)
                        else:
                            sg = shp.tile([P, GROUP], dt.float32, tag=f"sg{j}")
                            nc.scalar.activation(out=sg[:], in_=hps[:],
                                                 func=AF.Sigmoid,
                                                 bias=b1[:, j:j + 1], scale=1.0)
                            u = shp.tile([P, GROUP], dt.float32, tag=f"u{j}")
                            nc.vector.tensor_scalar(
                                out=u[:], in0=hps[:], scalar1=b1[:, j:j + 1],
                                scalar2=None, op0=ALU.add)
                            nc.vector.tensor_tensor(out=sh[:], in0=u[:],
                                                    in1=sg[:], op=ALU.mult)
                        sh_tiles.append(sh)

                    yps = psy.tile([P, GROUP], dt.float32, tag="yps")
                    for j in range(4):
                        nc.tensor.matmul(out=yps[:],
                                         lhsT=w2[:, j * 128:(j + 1) * 128],
                                         rhs=sh_tiles[j][:],
                                         start=(j == 0), stop=(j == 3))
                    y = yp.tile([P, GROUP], dt.bfloat16, tag="y")
                    nc.vector.tensor_scalar(out=y[:], in0=yps[:],
                                            scalar1=b2[:, 0:1], scalar2=None,
                                            op0=ALU.add)
                    y_tiles[g] = y
                    y2 = y2p.tile([P, GROUP], dt.bfloat16, tag="y2")
                    nc.vector.tensor_tensor(out=y2[:], in0=y[:], in1=y[:],
                                            op=ALU.mult)
                    y2_tiles[g] = y2
                # block-end stats burst (keeps stats matmuls off the
                # per-group PE critical path)
                for gi, g in enumerate(block):
                    onc_g = onb[:, gi * 128:(gi + 1) * 128]
                    nc.tensor.matmul(out=mu_ps[:], lhsT=onc_g,
                                     rhs=y_tiles[g][:],
                                     start=(gi == 0), stop=(gi == bsz - 1),
                                     skip_group_check=True)
                    nc.tensor.matmul(out=m2_ps[:], lhsT=onc_g,
                                     rhs=y2_tiles.pop(g)[:],
                                     start=(gi == 0), stop=(gi == bsz - 1),
                                     skip_group_check=True)
                _last_stats.append((mu_ps, m2_ps))

            def phase2(block, bi, mu_ps, m2_ps):
                mu_bf = stp.tile([P, GROUP], dt.bfloat16, tag="mubf")
                nc.scalar.activation(out=mu_bf[:], in_=mu_ps[:], func=AF.Copy)
                m2_bf = stp.tile([P, GROUP], dt.bfloat16, tag="m2bf")
                nc.scalar.activation(out=m2_bf[:], in_=m2_ps[:], func=AF.Copy)
                musq = stp.tile([P, GROUP], dt.bfloat16, tag="musq")
                nc.scalar.square(out=musq[:], in_=mu_bf[:])
                var = stp.tile([P, GROUP], dt.bfloat16, tag="var")
                nc.vector.tensor_tensor(out=var[:], in0=m2_bf[:], in1=musq[:],
                                        op=ALU.subtract)
                lnv = stp.tile([P, GROUP], dt.bfloat16, tag="lnv")
                nc.scalar.activation(out=lnv[:], in_=var[:], func=AF.Ln,
                                     bias=eps[:, 0:1], scale=1.0)
                rstd = stp.tile([P, GROUP], dt.bfloat16, tag="rstd")
                nc.scalar.activation(out=rstd[:], in_=lnv[:], func=AF.Exp,
                                     bias=0.0, scale=-0.5)
                bounce = drp.tile([len(block), 1024], dt.bfloat16, tag="bounce")
                nc.gpsimd.dma_start(out=bounce[:, 0:512],
                                    in_=mu_bf[0:len(block), :])
                nc.gpsimd.dma_start(out=bounce[:, 512:1024],
                                    in_=rstd[0:len(block), :])
                return bounce

            def phase3_group(g, gi, bounce):
                    nsl = slice(g * GROUP, (g + 1) * GROUP)
                    mr = zp.tile([P, 1024], dt.bfloat16, tag="mr")
                    bsl = bounce[gi:gi + 1, 0:1024]
                    nc.gpsimd.dma_start(out=mr[:], in_=bass.AP(
                        tensor=bsl.tensor, offset=bsl.offset,
                        ap=[[0, P], bsl.ap[1]]))
                    y = y_tiles.pop(g)
                    xtn = xtn_tiles.pop(g)
                    za = zp.tile([P, GROUP], dt.bfloat16, tag="za")
                    nc.vector.tensor_tensor(out=za[:], in0=y[:],
                                            in1=mr[:, 0:512], op=ALU.subtract)
                    zb = zp.tile([P, GROUP], dt.bfloat16, tag="zb")
                    nc.vector.tensor_tensor(out=zb[:], in0=za[:],
                                            in1=mr[:, 512:1024], op=ALU.mult)
                    zc = zp.tile([P, GROUP], dt.bfloat16, tag="zc")
                    nc.vector.tensor_scalar(out=zc[:], in0=zb[:],
                                            scalar1=gam[:, 0:1],
                                            scalar2=bet[:, 0:1],
                                            op0=ALU.mult, op1=ALU.add)
                    of = zp.tile([P, GROUP], dt.bfloat16, tag="of")
                    nc.vector.tensor_tensor(out=of[:], in0=zc[:], in1=xtn[:],
                                            op=ALU.add)
                    nc.gpsimd.dma_start(out=OUT[:, nsl], in_=of[:])

            # emission: P1(b) P2(b) P3(b). P3 is DVE+DMA-only; with the
            # stats burst at block end, P1(b+1)'s PE work has no DVE
            # dependencies that queue behind P3(b)'s chains.
            for bi, block in enumerate(blocks):
                phase1(block, bi)
                mu_ps, m2_ps = _last_stats.pop()
                bounce = phase2(block, bi, mu_ps, m2_ps)
                for gi, g in enumerate(block):
                    phase3_group(g, gi, bounce)

    nc.compile()
    return nc


# --------------------------------------------------------------------------
# host-side sharding / packing
# --------------------------------------------------------------------------

def _preprocess(inputs, n_cores, nodes_per_core):
    nf = np.ascontiguousarray(np.asarray(inputs["node_features"], np.float32))
    ef = np.ascontiguousarray(np.asarray(inputs["edge_features"], np.float32))
    src = np.asarray(inputs["src_indices"]).astype(np.int64)
    W1 = np.asarray(inputs["W1"], np.float32)
    b1 = np.asarray(inputs["b1"], np.float32)
    W2 = np.asarray(inputs["W2"], np.float32)
    b2 = np.asarray(inputs["b2"], np.float32)
    gam = np.asarray(inputs["ln_gamma"], np.float32)
    bet = np.asarray(inputs["ln_beta"], np.float32)

    n_nodes, d = nf.shape
    n_edges = ef.shape[0]
    tiles_per_core = nodes_per_core // P
    n_groups = nodes_per_core // GROUP
    if n_groups >= 8:
        bmax = n_groups - 4
    else:
        bmax = n_groups

    order = np.argsort(src, kind="stable")
    snode = src[order]
    core = snode // nodes_per_core
    tile_in_core = (snode % nodes_per_core) // P
    lid = snode % P
    pt = core * tiles_per_core + tile_in_core
    counts = np.bincount(pt, minlength=n_cores * tiles_per_core)
    # per-position chunk counts, shared across cores (SPMD uniformity)
    ccounts = np.ceil(counts.reshape(n_cores, tiles_per_core) / P).astype(int)
    cis = np.maximum(ccounts.max(axis=0), 1)
    coff = np.concatenate([[0], np.cumsum(cis)]).astype(int)
    ch = int(coff[-1])
    cmaxt = int(cis.max())

    starts = np.zeros(n_cores * tiles_per_core, np.int64)
    np.cumsum(counts[:-1], out=starts[1:])
    rank = np.arange(n_edges, dtype=np.int64) - starts[pt]
    chunk = rank // P
    p = rank % P
    cg = coff[tile_in_core] + chunk
    row = core * (P * ch) + p * ch + cg

    ebuf = np.zeros((n_cores * P * ch, d), np.float32)
    ebuf[row] = ef[order]
    EB8 = ebuf.reshape(n_cores, P, ch * d).astype(BF16).view(np.uint8)
    ohbuf = np.zeros((n_cores * P * ch, 128), FP8)
    ohbuf[row, lid] = 1.0
    OH8 = ohbuf.reshape(n_cores, P, ch * 128).view(np.uint8)
    parts = []
    for ti in range(tiles_per_core):
        a, b = int(coff[ti]), int(coff[ti + 1])
        parts.append(EB8[:, :, a * 256:b * 256])
        parts.append(OH8[:, :, a * 128:b * 128])
    PKa = np.ascontiguousarray(np.concatenate(parts, axis=2))

    nfp = np.zeros((n_cores * nodes_per_core, d), np.float32)
    nfp[:n_nodes] = nf
    NTBa = np.ascontiguousarray(
        nfp.reshape(n_cores, nodes_per_core, d).transpose(0, 2, 1)).astype(BF16)

    W1P = np.ascontiguousarray(
        W1.reshape(2, P, 4, P).transpose(1, 0, 2, 3).reshape(P, 1024)).astype(BF16)
    W2P = np.ascontiguousarray(
        W2.reshape(4, P, P).transpose(1, 0, 2).reshape(P, 512)).astype(BF16)
    B1P = np.ascontiguousarray(b1.reshape(4, P).T)
    B2P = np.ascontiguousarray(b2.reshape(P, 1))
    GAMP = np.ascontiguousarray(gam.reshape(P, 1))
    BETP = np.ascontiguousarray(bet.reshape(P, 1))
    ONB = np.zeros((P, bmax * 128), np.float32)
    for g in range(bmax):
        ONB[:, g * 128 + g] = 1.0 / P
    ONB = ONB.astype(BF16)

    in_maps = []
    for k in range(n_cores):
        in_maps.append({
            "pk": PKa[k], "ntb": NTBa[k],
            "w1p": W1P, "w2p": W2P, "b1p": B1P, "b2p": B2P,
            "gam": GAMP, "bet": BETP, "onb": ONB,
        })
    return in_maps, tuple(int(c) for c in cis)


def _assemble(results, n_nodes, n_cores, nodes_per_core):
    outs = np.stack([np.asarray(r["out"]) for r in results])
    full = outs.astype(np.float32).transpose(0, 2, 1).reshape(
        n_cores * nodes_per_core, -1)
    return np.ascontiguousarray(full[:n_nodes])


# --------------------------------------------------------------------------
# public entry point
# --------------------------------------------------------------------------

ACT_MODE = "silu"

_AXON_SO = "/opt/axon/libaxon_pjrt.so"


def _ensure_ntff_hook():
    """Provide antenv.axon_hooks + register the ctypes NTFF profile hook
    (the agent image's antenv lacks axon_hooks, so boot degraded silently)."""
    import sys
    import types
    import ctypes
    import contextlib
    import os

    try:
        from antenv.axon_hooks import get_axon_ntff_profile_hook  # noqa: F401
        return
    except ImportError:
        pass
    import antenv

    m = types.ModuleType("antenv.axon_hooks")
    m._hook = None

    def set_axon_ntff_profile_hook(h):
        m._hook = h

    def get_axon_ntff_profile_hook():
        return m._hook

    m.set_axon_ntff_profile_hook = set_axon_ntff_profile_hook
    m.get_axon_ntff_profile_hook = get_axon_ntff_profile_hook
    sys.modules["antenv.axon_hooks"] = m
    antenv.axon_hooks = m

    if not os.path.exists(_AXON_SO):
        return
    lib = ctypes.CDLL(_AXON_SO)
    if not hasattr(lib, "axon_start_nrt_profile"):
        return
    lib.axon_start_nrt_profile.argtypes = [ctypes.POINTER(ctypes.c_int64),
                                           ctypes.c_size_t]
    lib.axon_start_nrt_profile.restype = ctypes.c_int64
    lib.axon_stop_nrt_profile.argtypes = [ctypes.c_char_p]
    lib.axon_stop_nrt_profile.restype = ctypes.c_int64

    @contextlib.contextmanager
    def _hook(output_dir, device_ids):
        import jax

        jax.devices()
        if device_ids:
            ids = (ctypes.c_int64 * len(device_ids))(*device_ids)
            rc = lib.axon_start_nrt_profile(ids, len(device_ids))
        else:
            rc = lib.axon_start_nrt_profile(None, 0)
        if rc != 0:
            raise RuntimeError(f"axon_start_nrt_profile rc={rc}")
        try:
            yield
        finally:
            n = lib.axon_stop_nrt_profile(str(output_dir).encode())
            if n < 0:
                raise RuntimeError(f"axon_stop_nrt_profile rc={n}")
            if n == 0:
                print("WARNING: NTFF capture wrote no files")

    m._hook = _hook


def _run(inputs, trace=False):
    if trace:
        _ensure_ntff_hook()
    n_nodes = np.asarray(inputs["node_features"]).shape[0]
    in_maps, cis = _preprocess(inputs, N_CORES, NODES_PER_CORE)
    nc = _build(NODES_PER_CORE, cis, N_CORES, ACT_MODE)
    res = bass_utils.run_bass_kernel_spmd(
        nc, in_maps, core_ids=list(range(N_CORES)), trace=trace)
    out = _assemble(res.results, n_nodes, N_CORES, NODES_PER_CORE)
    return out, res


def kernel(**inputs):
    out, _ = _run(inputs, trace=False)
    return out


def kernel_profiled(**inputs):
    out, res = _run(inputs, trace=True)
    return out, res



# revision 3
# speedup vs baseline: 1.0816x; 1.0816x over previous
"""Trainium2 Bass kernel for nn_MeshNodeBlock (GNN message passing block).

reference semantics:
    agg = segment_sum(edge_features, src_indices, N)        # scatter-add
    x   = concat([node_features, agg], -1)
    h   = silu(x @ W1 + b1)
    y   = h @ W2 + b2
    y   = layer_norm(y) * gamma + beta
    out = y + node_features

Strategy (8 NeuronCores, SPMD, one NEFF):
  * Host assigns 128-node tiles to cores with a serpentine deal on per-tile
    chunk counts (load balance), sorts each core's tiles by count so the
    shared per-position chunk budget cis[i] = max over cores is tight.
  * Edges ship as bf16 feature rows plus ONE byte per edge (the local node
    id); the fp8 one-hot needed for the scatter matmuls is built on device
    with a single DVE is_equal per tile (iota bytes vs lids, broadcast APs).
  * Device works in transposed space (features on partitions, nodes free).
    Scatter-add per 128-node tile = ci PE matmuls aggT += edgeT @ onehot.
  * The layer-norm mean is folded into W2 on the host (W2c = W2 @ (I-J/128),
    b2c = b2 - mean(b2)), so layer 2 directly produces the mean-centered
    z0; variance is then just mean(z0^2) via one block-accumulated stats
    matmul per group, rstd = exp(-0.5 ln(var+eps)) once per block, and the
    per-node rstd row is partition-broadcast back via a tiny SWDGE DMA.
  * Normalize tail is 2 fused DVE ops per group:
    zb = (z0*gamma)*rstd_bc ; out = (zb+beta)+node, written bf16.
  * Emission is software-pipelined one group ahead (scatter of g+1 before
    MLP of g) so the PE never waits on the PSUM->SBUF agg copy or DMAs.
"""

import functools
from contextlib import ExitStack

import numpy as np
import ml_dtypes

import concourse.bass as bass
import concourse.tile as tile
from concourse import bacc, mybir
from concourse import bass_utils

BF16 = ml_dtypes.bfloat16
FP8 = ml_dtypes.float8_e4m3

N_NODES = 100000
D = 128
N_CORES = 8
P = 128
GROUP = 512              # nodes per group = 4 tiles
TILES_PER_CORE = 100
NODES_PER_CORE = TILES_PER_CORE * P   # 12800, 25 groups
EPS = 1e-5

AF = mybir.ActivationFunctionType
ALU = mybir.AluOpType
dt = mybir.dt


# --------------------------------------------------------------------------
# device kernel builder
# --------------------------------------------------------------------------

@functools.lru_cache(maxsize=4)
def _build(nodes_per_core: int, cis: tuple, n_cores: int):
    assert nodes_per_core % GROUP == 0
    n_groups = nodes_per_core // GROUP
    tiles_per_core = nodes_per_core // P
    assert len(cis) == tiles_per_core

    # per-position byte layout inside pk: 256*ci edge bytes + padded lids
    lidb = [4 * ((c + 3) // 4) for c in cis]
    segb = [256 * c + l for c, l in zip(cis, lidb)]
    boff = np.concatenate([[0], np.cumsum(segb)]).astype(int)
    pk_bytes = int(boff[-1])
    gbytes_max = max(int(boff[g * 4 + 4] - boff[g * 4]) for g in range(n_groups))
    gci = [sum(cis[g * 4:g * 4 + 4]) for g in range(n_groups)]
    gci_max = max(gci)

    # phase blocks of groups. Asymmetric: big first block, small last block.
    if n_groups >= 8:
        ntail = 4
        blocks = [list(range(0, n_groups - ntail)),
                  list(range(n_groups - ntail, n_groups))]
    else:
        blocks = [list(range(n_groups))]
    bmax = max(len(b) for b in blocks)

    nc = bacc.Bacc("TRN2", target_bir_lowering=False, debug=False,
                   enable_asserts=False, num_devices=n_cores)

    PK = nc.dram_tensor("pk", [P, pk_bytes], dt.uint8, kind="ExternalInput").ap()
    NTB = nc.dram_tensor("ntb", [P, nodes_per_core], dt.bfloat16,
                         kind="ExternalInput").ap()
    W1P = nc.dram_tensor("w1p", [P, 1024], dt.bfloat16, kind="ExternalInput").ap()
    W2P = nc.dram_tensor("w2p", [P, 512], dt.bfloat16, kind="ExternalInput").ap()
    B1P = nc.dram_tensor("b1p", [P, 4], dt.float32, kind="ExternalInput").ap()
    B2P = nc.dram_tensor("b2p", [P, 1], dt.float32, kind="ExternalInput").ap()
    GAM = nc.dram_tensor("gam", [P, 1], dt.float32, kind="ExternalInput").ap()
    BET = nc.dram_tensor("bet", [P, 1], dt.float32, kind="ExternalInput").ap()
    ONB = nc.dram_tensor("onb", [P, bmax * 128], dt.bfloat16,
                         kind="ExternalInput").ap()
    IOT = nc.dram_tensor("iot", [P, P], dt.uint8, kind="ExternalInput").ap()
    OUT = nc.dram_tensor("out", [P, nodes_per_core], dt.bfloat16,
                         kind="ExternalOutput").ap()

    with tile.TileContext(nc) as tc:
        with ExitStack() as ctx:
            singles = ctx.enter_context(tc.tile_pool(name="singles", bufs=1))
            ebp = ctx.enter_context(tc.tile_pool(name="ebp", bufs=4))
            ohp = ctx.enter_context(tc.tile_pool(name="ohp", bufs=3))
            xtp = ctx.enter_context(tc.tile_pool(name="xtp", bufs=n_groups + 2))
            xap = ctx.enter_context(tc.tile_pool(name="xap", bufs=4))
            shp = ctx.enter_context(tc.tile_pool(name="shp", bufs=2))
            z0p = ctx.enter_context(tc.tile_pool(name="z0p", bufs=n_groups + 2))
            sqp = ctx.enter_context(tc.tile_pool(name="sqp", bufs=3))
            rsp = ctx.enter_context(tc.tile_pool(name="rsp", bufs=2))
            bcp = ctx.enter_context(tc.tile_pool(name="bcp", bufs=4))
            ofp = ctx.enter_context(tc.tile_pool(name="ofp", bufs=4))
            psagg = ctx.enter_context(tc.tile_pool(name="psagg", bufs=2, space="PSUM"))
            psh = ctx.enter_context(tc.tile_pool(name="psh", bufs=2, space="PSUM"))
            psz = ctx.enter_context(tc.tile_pool(name="psz", bufs=2, space="PSUM"))
            psst = ctx.enter_context(tc.tile_pool(name="psst", bufs=2, space="PSUM"))
            drp = ctx.enter_context(tc.tile_pool(name="drp", bufs=2, space="DRAM"))

            def load_const(name, src, shape, dtyp):
                t = singles.tile(shape, dtyp, tag=name)
                nc.sync.dma_start(out=t[:], in_=src)
                return t

            w1 = load_const("w1", W1P, [P, 1024], dt.bfloat16)
            w2 = load_const("w2", W2P, [P, 512], dt.bfloat16)
            b1 = load_const("b1", B1P, [P, 4], dt.float32)
            b2c = load_const("b2c", B2P, [P, 1], dt.float32)
            gam = load_const("gam", GAM, [P, 1], dt.float32)
            bet = load_const("bet", BET, [P, 1], dt.float32)
            onb = load_const("onb", ONB, [P, bmax * 128], dt.bfloat16)
            iot = load_const("iot", IOT, [P, P], dt.uint8)
            eps = singles.tile([P, 1], dt.float32, tag="eps")
            nc.vector.memset(eps[:], EPS)

            xtn_tiles = {}
            z0_tiles = {}
            agg_tiles = {}
            _stats = {}

            def scat(g):
                """DMA + one-hot build + scatter matmuls for group g."""
                nsl = slice(g * GROUP, (g + 1) * GROUP)
                xtn = xtp.tile([P, GROUP], dt.bfloat16, tag="xtn")
                nc.sync.dma_start(out=xtn[:], in_=NTB[:, nsl])
                xtn_tiles[g] = xtn

                g0 = int(boff[g * 4])
                gbytes = int(boff[g * 4 + 4]) - g0
                pk = ebp.tile([P, gbytes_max], dt.uint8, tag="pk")
                nc.sync.dma_start(out=pk[:, :gbytes], in_=PK[:, g0:g0 + gbytes])

                oh = ohp.tile([P, gci_max * 128], dt.float8e4, tag="oh")
                ohoff = 0
                chunks = []
                for t4 in range(4):
                    ti = g * 4 + t4
                    ci = int(cis[ti])
                    toff = int(boff[ti]) - g0
                    ebv = pk[:, toff:toff + ci * 256].bitcast(dt.bfloat16)
                    lsl = pk[:, toff + ci * 256:toff + ci * 256 + ci]
                    # one-hot for the whole tile in one is_equal:
                    # out[p, c, j] = (iota[p, j] == lid[p, c])
                    in0 = bass.AP(tensor=iot[:].tensor, offset=iot[:].offset,
                                  ap=[iot[:].ap[0], [0, ci], [1, P]])
                    in1 = bass.AP(tensor=lsl.tensor, offset=lsl.offset,
                                  ap=[lsl.ap[0], [1, ci], [0, P]])
                    osl = oh[:, ohoff * 128:(ohoff + ci) * 128]
                    out3 = bass.AP(tensor=osl.tensor, offset=osl.offset,
                                   ap=[osl.ap[0], [P, ci], [1, P]])
                    nc.vector.tensor_tensor(out=out3, in0=in0, in1=in1,
                                            op=ALU.is_equal)
                    chunks.append((ebv, ohoff, ci))
                    ohoff += ci

                agg_ps = psagg.tile([P, GROUP], dt.float32, tag="agg")
                for t4, (ebv, off0, ci) in enumerate(chunks):
                    for c in range(ci):
                        nc.tensor.matmul(
                            out=agg_ps[:, t4 * 128:(t4 + 1) * 128],
                            lhsT=ebv[:, c * 128:(c + 1) * 128],
                            rhs=oh[:, (off0 + c) * 128:(off0 + c + 1) * 128],
                            start=(c == 0), stop=(c == ci - 1))
                # PSUM -> SBUF bf16 copy, alternating engine
                xta = xap.tile([P, GROUP], dt.bfloat16, tag="xta")
                if g % 2 == 0:
                    nc.scalar.activation(out=xta[:], in_=agg_ps[:], func=AF.Copy)
                else:
                    nc.vector.tensor_copy(out=xta[:], in_=agg_ps[:])
                agg_tiles[g] = xta

            def mlp(g, gi, bi, bsz):
                xtn = xtn_tiles[g]
                xta = agg_tiles.pop(g)
                sh_tiles = []
                for j in range(4):
                    hps = psh.tile([P, GROUP], dt.float32, tag="hps")
                    nc.tensor.matmul(out=hps[:],
                                     lhsT=w1[:, j * 128:(j + 1) * 128],
                                     rhs=xtn[:], start=True, stop=False)
                    nc.tensor.matmul(
                        out=hps[:],
                        lhsT=w1[:, 512 + j * 128:512 + (j + 1) * 128],
                        rhs=xta[:], start=False, stop=True)
                    sh = shp.tile([P, GROUP], dt.bfloat16, tag=f"sh{j}")
                    nc.scalar.activation(out=sh[:], in_=hps[:], func=AF.Silu,
                                         bias=b1[:, j:j + 1], scale=1.0)
                    sh_tiles.append(sh)

                zps = psz.tile([P, GROUP], dt.float32, tag="zps")
                for j in range(4):
                    nc.tensor.matmul(out=zps[:],
                                     lhsT=w2[:, j * 128:(j + 1) * 128],
                                     rhs=sh_tiles[j][:],
                                     start=(j == 0), stop=(j == 3))
                z0 = z0p.tile([P, GROUP], dt.bfloat16, tag="z0")
                nc.vector.tensor_scalar(out=z0[:], in0=zps[:],
                                        scalar1=b2c[:, 0:1], scalar2=None,
                                        op0=ALU.add)
                z0_tiles[g] = z0
                sq = sqp.tile([P, GROUP], dt.bfloat16, tag="sq")
                nc.vector.tensor_tensor(out=sq[:], in0=z0[:], in1=z0[:],
                                        op=ALU.mult)
                if gi == 0:
                    _stats[bi] = psst.tile([P, GROUP], dt.float32, tag="m2",
                                           name=f"m2_{bi}")
                nc.tensor.matmul(out=_stats[bi][:],
                                 lhsT=onb[:, gi * 128:(gi + 1) * 128],
                                 rhs=sq[:],
                                 start=(gi == 0), stop=(gi == bsz - 1),
                                 skip_group_check=True)

            def phase2(bi, bsz):
                m2_ps = _stats.pop(bi)
                lnv = rsp.tile([P, GROUP], dt.bfloat16, tag="lnv")
                nc.scalar.activation(out=lnv[:], in_=m2_ps[:], func=AF.Ln,
                                     bias=eps[:, 0:1], scale=1.0)
                rstd = rsp.tile([P, GROUP], dt.bfloat16, tag="rstd")
                nc.scalar.activation(out=rstd[:], in_=lnv[:], func=AF.Exp,
                                     bias=0.0, scale=-0.5)
                bounce = drp.tile([bsz, GROUP], dt.bfloat16, tag="bounce")
                nc.gpsimd.dma_start(out=bounce[:], in_=rstd[0:bsz, :])
                return bounce

            def phase3(g, gi, bounce):
                nsl = slice(g * GROUP, (g + 1) * GROUP)
                bc = bcp.tile([P, GROUP], dt.bfloat16, tag="bc")
                bsl = bounce[gi:gi + 1, :]
                nc.gpsimd.dma_start(out=bc[:], in_=bass.AP(
                    tensor=bsl.tensor, offset=bsl.offset,
                    ap=[[0, P], bsl.ap[1]]))
                z0 = z0_tiles.pop(g)
                xtn = xtn_tiles.pop(g)
                zb = ofp.tile([P, GROUP], dt.bfloat16, tag="zb")
                nc.vector.scalar_tensor_tensor(out=zb[:], in0=z0[:],
                                               scalar=gam[:, 0:1], in1=bc[:],
                                               op0=ALU.mult, op1=ALU.mult)
                of = ofp.tile([P, GROUP], dt.bfloat16, tag="of")
                nc.vector.scalar_tensor_tensor(out=of[:], in0=zb[:],
                                               scalar=bet[:, 0:1], in1=xtn[:],
                                               op0=ALU.add, op1=ALU.add)
                nc.scalar.dma_start(out=OUT[:, nsl], in_=of[:])

            # --- emission: 1-group scatter skew + P3 interleave ---
            g_of = []
            for bi, block in enumerate(blocks):
                for gi, g in enumerate(block):
                    g_of.append((g, gi, bi, len(block)))

            p3_queue = []
            scat(g_of[0][0])
            for idx, (g, gi, bi, bsz) in enumerate(g_of):
                if idx + 1 < len(g_of):
                    scat(g_of[idx + 1][0])
                mlp(g, gi, bi, bsz)
                if gi == bsz - 1:
                    bounce = phase2(bi, bsz)
                    p3_queue.extend((gg, ggi, bounce)
                                    for ggi, gg in enumerate(blocks[bi]))
                    # drain pace: spread pending P3 over the remaining groups
                    remaining = len(g_of) - idx - 1
                    pace = (len(p3_queue) + max(remaining, 1) - 1) // max(remaining, 1)
                elif p3_queue:
                    for _ in range(min(pace, len(p3_queue))):
                        phase3(*p3_queue.pop(0))
            while p3_queue:
                phase3(*p3_queue.pop(0))

    nc.compile()
    return nc


# --------------------------------------------------------------------------
# host-side sharding / packing
# --------------------------------------------------------------------------

def _preprocess(inputs, n_cores, nodes_per_core):
    nf = np.ascontiguousarray(np.asarray(inputs["node_features"], np.float32))
    ef = np.ascontiguousarray(np.asarray(inputs["edge_features"], np.float32))
    src = np.asarray(inputs["src_indices"]).astype(np.int64)
    W1 = np.asarray(inputs["W1"], np.float32)
    b1 = np.asarray(inputs["b1"], np.float32)
    W2 = np.asarray(inputs["W2"], np.float32)
    b2 = np.asarray(inputs["b2"], np.float32)
    gam = np.asarray(inputs["ln_gamma"], np.float32)
    bet = np.asarray(inputs["ln_beta"], np.float32)

    n_nodes, d = nf.shape
    n_edges = ef.shape[0]
    tiles_per_core = nodes_per_core // P
    n_tiles = n_cores * tiles_per_core
    n_groups = nodes_per_core // GROUP
    bmax = (n_groups - 4) if n_groups >= 8 else n_groups

    tile_e = src // P
    lid_e = (src % P).astype(np.uint8)
    counts = np.bincount(tile_e, minlength=n_tiles)
    cnt = np.ceil(counts / P).astype(int)

    # serpentine deal of tiles (desc by chunk count) into cores, then sort
    # each core's tiles desc so the shared per-position budget is tight
    order_t = np.argsort(-cnt, kind="stable")
    assign = np.empty((n_cores, tiles_per_core), np.int64)
    for r in range(tiles_per_core):
        row = order_t[r * n_cores:(r + 1) * n_cores]
        if r % 2 == 1:
            row = row[::-1]
        assign[:, r] = row
    # each core's row is already desc by construction of order_t
    core_of_tile = np.empty(n_tiles, np.int64)
    pos_of_tile = np.empty(n_tiles, np.int64)
    for k in range(n_cores):
        core_of_tile[assign[k]] = k
        pos_of_tile[assign[k]] = np.arange(tiles_per_core)

    cis = np.maximum(
        np.max(cnt[assign], axis=0), 1).astype(int)  # [tiles_per_core]
    coff = np.concatenate([[0], np.cumsum(cis)]).astype(int)
    CH = int(coff[-1])

    # edge slot placement
    order = np.argsort(src, kind="stable")
    snode = src[order]
    stile = snode // P
    starts = np.zeros(n_tiles, np.int64)
    np.cumsum(counts[:-1], out=starts[1:])
    rank = np.arange(n_edges, dtype=np.int64) - starts[stile]
    chunk = rank // P
    part = rank % P
    score = core_of_tile[stile]
    spos = pos_of_tile[stile]
    cslot = coff[spos] + chunk

    earr = np.zeros((n_cores, CH, P, d), BF16)
    earr[score, cslot, part] = ef[order].astype(BF16)
    larr = np.full((n_cores, CH, P), 255, np.uint8)
    larr[score, cslot, part] = lid_e[order]

    # pack pk bytes: per position, edge seg then padded lid seg
    segs = []
    for i in range(tiles_per_core):
        a, b = int(coff[i]), int(coff[i + 1])
        ci = b - a
        eseg = np.ascontiguousarray(
            earr[:, a:b].transpose(0, 2, 1, 3)).reshape(
                n_cores, P, ci * d * 2 // 2)  # [ncores, P, ci*128] bf16
        segs.append(eseg.view(np.uint8).reshape(n_cores, P, ci * 256))
        lpad = 4 * ((ci + 3) // 4)
        lseg = np.full((n_cores, P, lpad), 255, np.uint8)
        lseg[:, :, :ci] = larr[:, a:b].transpose(0, 2, 1)
        segs.append(lseg)
    PKa = np.ascontiguousarray(np.concatenate(segs, axis=2))

    # node features packed in assigned-tile order, transposed, bf16
    nfp = np.zeros((n_tiles * P, d), np.float32)
    nfp[:n_nodes] = nf
    tiles_nf = nfp.reshape(n_tiles, P, d)
    NTBa = np.empty((n_cores, P, nodes_per_core), BF16)
    for k in range(n_cores):
        blk = tiles_nf[assign[k]].reshape(nodes_per_core, d)
        NTBa[k] = blk.T.astype(BF16)

    # fold layernorm mean-centering into W2 / b2
    W2c = W2 - W2.mean(axis=1, keepdims=True)
    b2c = (b2 - b2.mean()).astype(np.float32)

    W1P = np.ascontiguousarray(
        W1.reshape(2, P, 4, P).transpose(1, 0, 2, 3).reshape(P, 1024)).astype(BF16)
    W2P = np.ascontiguousarray(
        W2c.reshape(4, P, P).transpose(1, 0, 2).reshape(P, 512)).astype(BF16)
    B1P = np.ascontiguousarray(b1.reshape(4, P).T)
    B2P = np.ascontiguousarray(b2c.reshape(P, 1))
    GAMP = np.ascontiguousarray(gam.reshape(P, 1))
    BETP = np.ascontiguousarray(bet.reshape(P, 1))
    ONB = np.zeros((P, bmax * 128), np.float32)
    for g in range(bmax):
        ONB[:, g * 128 + g] = 1.0 / P
    ONB = ONB.astype(BF16)
    IOT = np.tile(np.arange(P, dtype=np.uint8), (P, 1))

    in_maps = []
    for k in range(n_cores):
        in_maps.append({
            "pk": PKa[k], "ntb": NTBa[k],
            "w1p": W1P, "w2p": W2P, "b1p": B1P, "b2p": B2P,
            "gam": GAMP, "bet": BETP, "onb": ONB, "iot": IOT,
        })
    return in_maps, tuple(int(c) for c in cis), assign


def _assemble(results, n_nodes, n_cores, nodes_per_core, assign):
    tiles_per_core = nodes_per_core // P
    n_tiles = n_cores * tiles_per_core
    full = np.empty((n_tiles, P, D), np.float32)
    for k in range(n_cores):
        outk = np.asarray(results[k]["out"]).astype(np.float32)  # [P, npc]
        full[assign[k]] = outk.T.reshape(tiles_per_core, P, D)
    return np.ascontiguousarray(full.reshape(n_tiles * P, D)[:n_nodes])


# --------------------------------------------------------------------------
# public entry point
# --------------------------------------------------------------------------

_AXON_SO = "/opt/axon/libaxon_pjrt.so"


def _ensure_ntff_hook():
    """Provide antenv.axon_hooks + register the ctypes NTFF profile hook
    (the agent image's antenv lacks axon_hooks, so boot degraded silently)."""
    import sys
    import types
    import ctypes
    import contextlib
    import os

    try:
        from antenv.axon_hooks import get_axon_ntff_profile_hook  # noqa: F401
        return
    except ImportError:
        pass
    import antenv

    m = types.ModuleType("antenv.axon_hooks")
    m._hook = None

    def set_axon_ntff_profile_hook(h):
        m._hook = h

    def get_axon_ntff_profile_hook():
        return m._hook

    m.set_axon_ntff_profile_hook = set_axon_ntff_profile_hook
    m.get_axon_ntff_profile_hook = get_axon_ntff_profile_hook
    sys.modules["antenv.axon_hooks"] = m
    antenv.axon_hooks = m

    if not os.path.exists(_AXON_SO):
        return
    lib = ctypes.CDLL(_AXON_SO)
    if not hasattr(lib, "axon_start_nrt_profile"):
        return
    lib.axon_start_nrt_profile.argtypes = [ctypes.POINTER(ctypes.c_int64),
                                           ctypes.c_size_t]
    lib.axon_start_nrt_profile.restype = ctypes.c_int64
    lib.axon_stop_nrt_profile.argtypes = [ctypes.c_char_p]
    lib.axon_stop_nrt_profile.restype = ctypes.c_int64

    @contextlib.contextmanager
    def _hook(output_dir, device_ids):
        import jax

        jax.devices()
        if device_ids:
            ids = (ctypes.c_int64 * len(device_ids))(*device_ids)
            rc = lib.axon_start_nrt_profile(ids, len(device_ids))
        else:
            rc = lib.axon_start_nrt_profile(None, 0)
        if rc != 0:
            raise RuntimeError(f"axon_start_nrt_profile rc={rc}")
        try:
            yield
        finally:
            n = lib.axon_stop_nrt_profile(str(output_dir).encode())
            if n < 0:
                raise RuntimeError(f"axon_stop_nrt_profile rc={n}")
            if n == 0:
                print("WARNING: NTFF capture wrote no files")

    m._hook = _hook


def _run(inputs, trace=False):
    if trace:
        _ensure_ntff_hook()
    n_nodes = np.asarray(inputs["node_features"]).shape[0]
    in_maps, cis, assign = _preprocess(inputs, N_CORES, NODES_PER_CORE)
    nc = _build(NODES_PER_CORE, cis, N_CORES)
    res = bass_utils.run_bass_kernel_spmd(
        nc, in_maps, core_ids=list(range(N_CORES)), trace=trace)
    out = _assemble(res.results, n_nodes, N_CORES, NODES_PER_CORE, assign)
    return out, res


def kernel(**inputs):
    out, _ = _run(inputs, trace=False)
    return out


def kernel_profiled(**inputs):
    out, res = _run(inputs, trace=True)
    return out, res


# revision 12
# speedup vs baseline: 1.1534x; 1.0664x over previous
"""Trainium2 Bass kernel for nn_MeshNodeBlock (GNN message passing block).

reference semantics:
    agg = segment_sum(edge_features, src_indices, N)        # scatter-add
    x   = concat([node_features, agg], -1)
    h   = silu(x @ W1 + b1)
    y   = h @ W2 + b2
    y   = layer_norm(y) * gamma + beta
    out = y + node_features

Strategy (8 NeuronCores, SPMD, one NEFF):
  * Host assigns 128-node tiles to cores with a serpentine deal on per-tile
    chunk counts (load balance), sorts each core's tiles by count so the
    shared per-position chunk budget cis[i] = max over cores is tight.
  * Edges ship as bf16 feature rows plus a WIDTH-64 fp8 one-hot row (the
    128-node tile is split into two 64-node subtiles), 320B per edge slot;
    this keeps the DVE free of one-hot building at modest DMA cost.
  * Device works in transposed space (features on partitions, nodes free).
    Scatter-add per 128-node tile = ci PE matmuls aggT += edgeT @ onehot.
  * The layer-norm mean is folded into W2 on the host (W2c = W2 @ (I-J/128),
    b2c = b2 - mean(b2)), so layer 2 directly produces the mean-centered
    z0; variance is then just mean(z0^2) via one block-accumulated stats
    matmul per group, rstd = exp(-0.5 ln(var+eps)) once per block, and the
    per-node rstd row is partition-broadcast back via a tiny SWDGE DMA.
  * Normalize tail is 2 fused DVE ops per group:
    zb = (z0*gamma)*rstd_bc ; out = (zb+beta)+node, written bf16.
  * Emission is software-pipelined one group ahead (scatter of g+1 before
    MLP of g) so the PE never waits on the PSUM->SBUF agg copy or DMAs.
"""

import functools
from contextlib import ExitStack

import numpy as np
import ml_dtypes

import concourse.bass as bass
import concourse.tile as tile
from concourse import bacc, mybir
from concourse import bass_utils

BF16 = ml_dtypes.bfloat16
FP8 = ml_dtypes.float8_e4m3

N_NODES = 100000
D = 128
N_CORES = 8
P = 128
GROUP = 512              # nodes per group = 4 tiles
TILES_PER_CORE = 100
NODES_PER_CORE = TILES_PER_CORE * P   # 12800, 25 groups
EPS = 1e-5

AF = mybir.ActivationFunctionType
ALU = mybir.AluOpType
dt = mybir.dt


# --------------------------------------------------------------------------
# device kernel builder
# --------------------------------------------------------------------------

@functools.lru_cache(maxsize=4)
def _build(nodes_per_core: int, cis: tuple, n_cores: int):
    """cis: per tile position, (cA, cB) chunk budgets for the two subtiles."""
    assert nodes_per_core % GROUP == 0
    n_groups = nodes_per_core // GROUP
    tiles_per_core = nodes_per_core // P
    assert len(cis) == tiles_per_core

    # per-position byte layout inside pk: (cA+cB)*256 edge bytes + *64 onehot
    segb = [320 * (ca + cb) for ca, cb in cis]
    boff = np.concatenate([[0], np.cumsum(segb)]).astype(int)
    pk_bytes = int(boff[-1])
    gbytes_max = max(int(boff[g * 4 + 4] - boff[g * 4]) for g in range(n_groups))

    # phase blocks of groups. Asymmetric: big first block, small last block.
    if n_groups >= 8:
        ntail = 4
        blocks = [list(range(0, n_groups - ntail)),
                  list(range(n_groups - ntail, n_groups))]
    else:
        blocks = [list(range(n_groups))]
    bmax = max(len(b) for b in blocks)

    nc = bacc.Bacc("TRN2", target_bir_lowering=False, debug=False,
                   enable_asserts=False, num_devices=n_cores)

    PK = nc.dram_tensor("pk", [P, pk_bytes], dt.uint8, kind="ExternalInput").ap()
    NTB = nc.dram_tensor("ntb", [P, nodes_per_core], dt.bfloat16,
                         kind="ExternalInput").ap()
    W1P = nc.dram_tensor("w1p", [P, 1024], dt.bfloat16, kind="ExternalInput").ap()
    W2P = nc.dram_tensor("w2p", [P, 512], dt.bfloat16, kind="ExternalInput").ap()
    B1P = nc.dram_tensor("b1p", [P, 4], dt.float32, kind="ExternalInput").ap()
    B2P = nc.dram_tensor("b2p", [P, 1], dt.float32, kind="ExternalInput").ap()
    GAM = nc.dram_tensor("gam", [P, 1], dt.float32, kind="ExternalInput").ap()
    BET = nc.dram_tensor("bet", [P, 1], dt.float32, kind="ExternalInput").ap()
    ONB = nc.dram_tensor("onb", [P, bmax * 128], dt.bfloat16,
                         kind="ExternalInput").ap()
    OUT = nc.dram_tensor("out", [P, nodes_per_core], dt.bfloat16,
                         kind="ExternalOutput").ap()

    with tile.TileContext(nc) as tc:
        with ExitStack() as ctx:
            singles = ctx.enter_context(tc.tile_pool(name="singles", bufs=1))
            ebp = ctx.enter_context(tc.tile_pool(name="ebp", bufs=4))
            xtp = ctx.enter_context(tc.tile_pool(name="xtp", bufs=n_groups + 2))
            xap = ctx.enter_context(tc.tile_pool(name="xap", bufs=4))
            shp = ctx.enter_context(tc.tile_pool(name="shp", bufs=2))
            z0p = ctx.enter_context(tc.tile_pool(name="z0p", bufs=n_groups + 2))
            sqp = ctx.enter_context(tc.tile_pool(name="sqp", bufs=3))
            rsp = ctx.enter_context(tc.tile_pool(name="rsp", bufs=2))
            bcp = ctx.enter_context(tc.tile_pool(name="bcp", bufs=4))
            ofp = ctx.enter_context(tc.tile_pool(name="ofp", bufs=4))
            psagg = ctx.enter_context(tc.tile_pool(name="psagg", bufs=2, space="PSUM"))
            psh = ctx.enter_context(tc.tile_pool(name="psh", bufs=2, space="PSUM"))
            psz = ctx.enter_context(tc.tile_pool(name="psz", bufs=2, space="PSUM"))
            psst = ctx.enter_context(tc.tile_pool(name="psst", bufs=2, space="PSUM"))
            drp = ctx.enter_context(tc.tile_pool(name="drp", bufs=2, space="DRAM"))

            def load_const(name, src, shape, dtyp):
                t = singles.tile(shape, dtyp, tag=name)
                nc.scalar.dma_start(out=t[:], in_=src)
                return t

            w1 = load_const("w1", W1P, [P, 1024], dt.bfloat16)
            w2 = load_const("w2", W2P, [P, 512], dt.bfloat16)
            b1 = load_const("b1", B1P, [P, 4], dt.float32)
            b2c = load_const("b2c", B2P, [P, 1], dt.float32)
            gam = load_const("gam", GAM, [P, 1], dt.float32)
            bet = load_const("bet", BET, [P, 1], dt.float32)
            onb = load_const("onb", ONB, [P, bmax * 128], dt.bfloat16)
            eps = singles.tile([P, 1], dt.float32, tag="eps")
            nc.vector.memset(eps[:], EPS)

            xtn_tiles = {}
            z0_tiles = {}
            agg_tiles = {}
            _stats = {}

            def scat(g):
                """DMA + scatter matmuls for group g."""
                nsl = slice(g * GROUP, (g + 1) * GROUP)
                xtn = xtp.tile([P, GROUP], dt.bfloat16, tag="xtn")
                nc.scalar.dma_start(out=xtn[:], in_=NTB[:, nsl])
                xtn_tiles[g] = xtn

                g0 = int(boff[g * 4])
                gbytes = int(boff[g * 4 + 4]) - g0
                pk = ebp.tile([P, gbytes_max], dt.uint8, tag="pk")
                nc.sync.dma_start(out=pk[:, :gbytes], in_=PK[:, g0:g0 + gbytes])

                agg_ps = psagg.tile([P, GROUP], dt.float32, tag="agg")
                for t4 in range(4):
                    ti = g * 4 + t4
                    ca, cb = cis[ti]
                    ct = ca + cb
                    toff = int(boff[ti]) - g0
                    ebv = pk[:, toff:toff + ct * 256].bitcast(dt.bfloat16)
                    ohv = pk[:, toff + ct * 256:toff + ct * 320].bitcast(
                        dt.float8e4)
                    for sub, (c0, cn) in enumerate(((0, ca), (ca, cb))):
                        for k in range(cn):
                            c = c0 + k
                            nc.tensor.matmul(
                                out=agg_ps[:, t4 * 128 + sub * 64:
                                           t4 * 128 + sub * 64 + 64],
                                lhsT=ebv[:, c * 128:(c + 1) * 128],
                                rhs=ohv[:, c * 64:(c + 1) * 64],
                                start=(k == 0), stop=(k == cn - 1))
                # PSUM -> SBUF bf16 copy, alternating engine
                xta = xap.tile([P, GROUP], dt.bfloat16, tag="xta")
                if g % 2 == 0:
                    nc.scalar.activation(out=xta[:], in_=agg_ps[:], func=AF.Copy)
                else:
                    nc.vector.tensor_copy(out=xta[:], in_=agg_ps[:])
                agg_tiles[g] = xta

            def mlp(g, gi, bi, bsz):
                xtn = xtn_tiles[g]
                xta = agg_tiles.pop(g)
                sh_tiles = []
                for j in range(4):
                    hps = psh.tile([P, GROUP], dt.float32, tag="hps")
                    nc.tensor.matmul(out=hps[:],
                                     lhsT=w1[:, j * 128:(j + 1) * 128],
                                     rhs=xtn[:], start=True, stop=False)
                    nc.tensor.matmul(
                        out=hps[:],
                        lhsT=w1[:, 512 + j * 128:512 + (j + 1) * 128],
                        rhs=xta[:], start=False, stop=True)
                    sh = shp.tile([P, GROUP], dt.bfloat16, tag=f"sh{j}")
                    nc.scalar.activation(out=sh[:], in_=hps[:], func=AF.Silu,
                                         bias=b1[:, j:j + 1], scale=1.0)
                    sh_tiles.append(sh)

                zps = psz.tile([P, GROUP], dt.float32, tag="zps")
                for j in range(4):
                    nc.tensor.matmul(out=zps[:],
                                     lhsT=w2[:, j * 128:(j + 1) * 128],
                                     rhs=sh_tiles[j][:],
                                     start=(j == 0), stop=(j == 3))
                z0 = z0p.tile([P, GROUP], dt.bfloat16, tag="z0")
                nc.vector.tensor_scalar(out=z0[:], in0=zps[:],
                                        scalar1=b2c[:, 0:1], scalar2=None,
                                        op0=ALU.add)
                z0_tiles[g] = z0
                sq = sqp.tile([P, GROUP], dt.bfloat16, tag="sq")
                nc.vector.tensor_tensor(out=sq[:], in0=z0[:], in1=z0[:],
                                        op=ALU.mult)
                if gi == 0:
                    _stats[bi] = psst.tile([P, GROUP], dt.float32, tag="m2",
                                           name=f"m2_{bi}")
                nc.tensor.matmul(out=_stats[bi][:],
                                 lhsT=onb[:, gi * 128:(gi + 1) * 128],
                                 rhs=sq[:],
                                 start=(gi == 0), stop=(gi == bsz - 1),
                                 skip_group_check=True)

            def phase2(bi, bsz):
                m2_ps = _stats.pop(bi)
                lnv = rsp.tile([P, GROUP], dt.bfloat16, tag="lnv")
                nc.scalar.activation(out=lnv[:], in_=m2_ps[:], func=AF.Ln,
                                     bias=eps[:, 0:1], scale=1.0)
                rstd = rsp.tile([P, GROUP], dt.bfloat16, tag="rstd")
                nc.scalar.activation(out=rstd[:], in_=lnv[:], func=AF.Exp,
                                     bias=0.0, scale=-0.5)
                bounce = drp.tile([bsz, GROUP], dt.bfloat16, tag="bounce")
                nc.gpsimd.dma_start(out=bounce[:], in_=rstd[0:bsz, :])
                return bounce

            def phase3(g, gi, bounce):
                nsl = slice(g * GROUP, (g + 1) * GROUP)
                bc = bcp.tile([P, GROUP], dt.bfloat16, tag="bc")
                bsl = bounce[gi:gi + 1, :]
                nc.gpsimd.dma_start(out=bc[:], in_=bass.AP(
                    tensor=bsl.tensor, offset=bsl.offset,
                    ap=[[0, P], bsl.ap[1]]))
                z0 = z0_tiles.pop(g)
                xtn = xtn_tiles.pop(g)
                zb = ofp.tile([P, GROUP], dt.bfloat16, tag="zb")
                nc.vector.tensor_tensor(out=zb[:], in0=z0[:], in1=bc[:],
                                        op=ALU.mult)
                zc = ofp.tile([P, GROUP], dt.bfloat16, tag="zc")
                nc.vector.tensor_scalar(out=zc[:], in0=zb[:],
                                        scalar1=gam[:, 0:1],
                                        scalar2=bet[:, 0:1],
                                        op0=ALU.mult, op1=ALU.add)
                of = ofp.tile([P, GROUP], dt.bfloat16, tag="of")
                nc.vector.tensor_tensor(out=of[:], in0=zc[:], in1=xtn[:],
                                        op=ALU.add)
                nc.scalar.dma_start(out=OUT[:, nsl], in_=of[:])

            # --- emission: 1-group scatter skew + P3 interleave ---
            g_of = []
            for bi, block in enumerate(blocks):
                for gi, g in enumerate(block):
                    g_of.append((g, gi, bi, len(block)))

            p3_queue = []
            scat(g_of[0][0])
            for idx, (g, gi, bi, bsz) in enumerate(g_of):
                if idx + 1 < len(g_of):
                    scat(g_of[idx + 1][0])
                mlp(g, gi, bi, bsz)
                if gi == bsz - 1:
                    bounce = phase2(bi, bsz)
                    p3_queue.extend((gg, ggi, bounce)
                                    for ggi, gg in enumerate(blocks[bi]))
                    # drain pace: spread pending P3 over the remaining groups
                    remaining = len(g_of) - idx - 1
                    pace = (len(p3_queue) + max(remaining, 1) - 1) // max(remaining, 1)
                elif p3_queue:
                    for _ in range(min(pace, len(p3_queue))):
                        phase3(*p3_queue.pop(0))
            while p3_queue:
                phase3(*p3_queue.pop(0))

    nc.compile()
    return nc


# --------------------------------------------------------------------------
# host-side sharding / packing
# --------------------------------------------------------------------------

def _preprocess(inputs, n_cores, nodes_per_core):
    nf = np.ascontiguousarray(np.asarray(inputs["node_features"], np.float32))
    ef = np.ascontiguousarray(np.asarray(inputs["edge_features"], np.float32))
    src = np.asarray(inputs["src_indices"]).astype(np.int64)
    W1 = np.asarray(inputs["W1"], np.float32)
    b1 = np.asarray(inputs["b1"], np.float32)
    W2 = np.asarray(inputs["W2"], np.float32)
    b2 = np.asarray(inputs["b2"], np.float32)
    gam = np.asarray(inputs["ln_gamma"], np.float32)
    bet = np.asarray(inputs["ln_beta"], np.float32)

    n_nodes, d = nf.shape
    n_edges = ef.shape[0]
    tiles_per_core = nodes_per_core // P
    n_tiles = n_cores * tiles_per_core
    n_groups = nodes_per_core // GROUP
    bmax = (n_groups - 4) if n_groups >= 8 else n_groups

    # subtile = (tile, half); width-64 one-hot
    sub_e = src // 64                      # global subtile id, 2*n_tiles
    lid64 = (src % 64).astype(np.int64)
    scounts = np.bincount(sub_e, minlength=2 * n_tiles)
    scnt = np.ceil(scounts / P).astype(int).reshape(n_tiles, 2)

    # serpentine deal of tiles (desc by total chunk count) into cores, then
    # sort each core's tiles desc so the shared per-position budget is tight
    tot = scnt.sum(axis=1)
    order_t = np.argsort(-tot, kind="stable")
    assign = np.empty((n_cores, tiles_per_core), np.int64)
    for r in range(tiles_per_core):
        row = order_t[r * n_cores:(r + 1) * n_cores]
        if r % 2 == 1:
            row = row[::-1]
        assign[:, r] = row
    core_of_tile = np.empty(n_tiles, np.int64)
    pos_of_tile = np.empty(n_tiles, np.int64)
    for k in range(n_cores):
        core_of_tile[assign[k]] = k
        pos_of_tile[assign[k]] = np.arange(tiles_per_core)

    cisA = np.maximum(np.max(scnt[assign, 0], axis=0), 1).astype(int)
    cisB = np.maximum(np.max(scnt[assign, 1], axis=0), 1).astype(int)
    cist = cisA + cisB
    coff = np.concatenate([[0], np.cumsum(cist)]).astype(int)
    CH = int(coff[-1])

    # edge slot placement (within-subtile rank -> chunk, partition)
    order = np.argsort(src, kind="stable")
    snode = src[order]
    ssub = snode // 64
    sstarts = np.zeros(2 * n_tiles, np.int64)
    np.cumsum(scounts[:-1], out=sstarts[1:])
    rank = np.arange(n_edges, dtype=np.int64) - sstarts[ssub]
    chunk = rank // P
    part = rank % P
    stile = ssub // 2
    shalf = ssub % 2
    score = core_of_tile[stile]
    spos = pos_of_tile[stile]
    cslot = coff[spos] + np.where(shalf == 1, cisA[spos], 0) + chunk

    earr = np.zeros((n_cores, CH, P, d), BF16)
    earr[score, cslot, part] = ef[order].astype(BF16)
    oarr = np.zeros((n_cores, CH, P, 64), FP8)
    oarr[score, cslot, part, lid64[order]] = 1.0

    # pack pk bytes: per position, edge seg then one-hot seg
    segs = []
    for i in range(tiles_per_core):
        a, b = int(coff[i]), int(coff[i + 1])
        ci = b - a
        eseg = np.ascontiguousarray(
            earr[:, a:b].transpose(0, 2, 1, 3)).reshape(n_cores, P, ci * d)
        segs.append(eseg.view(np.uint8).reshape(n_cores, P, ci * 256))
        oseg = np.ascontiguousarray(
            oarr[:, a:b].transpose(0, 2, 1, 3)).reshape(n_cores, P, ci * 64)
        segs.append(oseg.view(np.uint8))
    PKa = np.ascontiguousarray(np.concatenate(segs, axis=2))

    # node features packed in assigned-tile order, transposed, bf16
    nfp = np.zeros((n_tiles * P, d), np.float32)
    nfp[:n_nodes] = nf
    tiles_nf = nfp.reshape(n_tiles, P, d)
    NTBa = np.empty((n_cores, P, nodes_per_core), BF16)
    for k in range(n_cores):
        blk = tiles_nf[assign[k]].reshape(nodes_per_core, d)
        NTBa[k] = blk.T.astype(BF16)

    # fold layernorm mean-centering into W2 / b2
    W2c = W2 - W2.mean(axis=1, keepdims=True)
    b2c = (b2 - b2.mean()).astype(np.float32)

    W1P = np.ascontiguousarray(
        W1.reshape(2, P, 4, P).transpose(1, 0, 2, 3).reshape(P, 1024)).astype(BF16)
    W2P = np.ascontiguousarray(
        W2c.reshape(4, P, P).transpose(1, 0, 2).reshape(P, 512)).astype(BF16)
    B1P = np.ascontiguousarray(b1.reshape(4, P).T)
    B2P = np.ascontiguousarray(b2c.reshape(P, 1))
    GAMP = np.ascontiguousarray(gam.reshape(P, 1))
    BETP = np.ascontiguousarray(bet.reshape(P, 1))
    ONB = np.zeros((P, bmax * 128), np.float32)
    for g in range(bmax):
        ONB[:, g * 128 + g] = 1.0 / P
    ONB = ONB.astype(BF16)

    in_maps = []
    for k in range(n_cores):
        in_maps.append({
            "pk": PKa[k], "ntb": NTBa[k],
            "w1p": W1P, "w2p": W2P, "b1p": B1P, "b2p": B2P,
            "gam": GAMP, "bet": BETP, "onb": ONB,
        })
    cis = tuple((int(a), int(b)) for a, b in zip(cisA, cisB))
    return in_maps, cis, assign


def _assemble(results, n_nodes, n_cores, nodes_per_core, assign):
    tiles_per_core = nodes_per_core // P
    n_tiles = n_cores * tiles_per_core
    full = np.empty((n_tiles, P, D), np.float32)
    for k in range(n_cores):
        outk = np.asarray(results[k]["out"]).astype(np.float32)  # [P, npc]
        full[assign[k]] = outk.T.reshape(tiles_per_core, P, D)
    return np.ascontiguousarray(full.reshape(n_tiles * P, D)[:n_nodes])


# --------------------------------------------------------------------------
# public entry point
# --------------------------------------------------------------------------

_AXON_SO = "/opt/axon/libaxon_pjrt.so"


def _ensure_ntff_hook():
    """Provide antenv.axon_hooks + register the ctypes NTFF profile hook
    (the agent image's antenv lacks axon_hooks, so boot degraded silently)."""
    import sys
    import types
    import ctypes
    import contextlib
    import os

    try:
        from antenv.axon_hooks import get_axon_ntff_profile_hook  # noqa: F401
        return
    except ImportError:
        pass
    import antenv

    m = types.ModuleType("antenv.axon_hooks")
    m._hook = None

    def set_axon_ntff_profile_hook(h):
        m._hook = h

    def get_axon_ntff_profile_hook():
        return m._hook

    m.set_axon_ntff_profile_hook = set_axon_ntff_profile_hook
    m.get_axon_ntff_profile_hook = get_axon_ntff_profile_hook
    sys.modules["antenv.axon_hooks"] = m
    antenv.axon_hooks = m

    if not os.path.exists(_AXON_SO):
        return
    lib = ctypes.CDLL(_AXON_SO)
    if not hasattr(lib, "axon_start_nrt_profile"):
        return
    lib.axon_start_nrt_profile.argtypes = [ctypes.POINTER(ctypes.c_int64),
                                           ctypes.c_size_t]
    lib.axon_start_nrt_profile.restype = ctypes.c_int64
    lib.axon_stop_nrt_profile.argtypes = [ctypes.c_char_p]
    lib.axon_stop_nrt_profile.restype = ctypes.c_int64

    @contextlib.contextmanager
    def _hook(output_dir, device_ids):
        import jax

        jax.devices()
        if device_ids:
            ids = (ctypes.c_int64 * len(device_ids))(*device_ids)
            rc = lib.axon_start_nrt_profile(ids, len(device_ids))
        else:
            rc = lib.axon_start_nrt_profile(None, 0)
        if rc != 0:
            raise RuntimeError(f"axon_start_nrt_profile rc={rc}")
        try:
            yield
        finally:
            n = lib.axon_stop_nrt_profile(str(output_dir).encode())
            if n < 0:
                raise RuntimeError(f"axon_stop_nrt_profile rc={n}")
            if n == 0:
                print("WARNING: NTFF capture wrote no files")

    m._hook = _hook


def _run(inputs, trace=False):
    if trace:
        _ensure_ntff_hook()
    n_nodes = np.asarray(inputs["node_features"]).shape[0]
    in_maps, cis, assign = _preprocess(inputs, N_CORES, NODES_PER_CORE)
    nc = _build(NODES_PER_CORE, cis, N_CORES)
    res = bass_utils.run_bass_kernel_spmd(
        nc, in_maps, core_ids=list(range(N_CORES)), trace=trace)
    out = _assemble(res.results, n_nodes, N_CORES, NODES_PER_CORE, assign)
    return out, res


def kernel(**inputs):
    out, _ = _run(inputs, trace=False)
    return out


def kernel_profiled(**inputs):
    out, res = _run(inputs, trace=True)
    return out, res
